# revision 9
# baseline (speedup 1.0000x reference)
"""MoE FFN kernel for 8 Trainium2 NeuronCores (expert-parallel).

Strategy:
  - Host computes the router (float64) and dispatches tokens by top-2
    assignment. Experts are sorted by token count and paired
    (rank i, rank 15-i) onto core i, so per-core work is balanced.
  - Each core runs 3 FFN instances: its two routed experts (hidden 1024)
    over their dispatched tokens, and the shared expert (hidden 2048) over
    a 1/8 token slice.  All matmuls in bf16 with fp32 PSUM accumulation;
    tokens live on the matmul free dim so no transposes are needed.
  - Gate weights are folded into the host-side scatter-add combine.
"""

import numpy as np
import ml_dtypes
from contextlib import ExitStack

import concourse.mybir as mybir
import concourse.tile as tile
from concourse import bacc
from concourse.bass_utils import run_bass_kernel_spmd

P = 128
D = 2048
H_E = 1024           # expert hidden dim
H_S = 2048           # shared expert hidden dim (EXPERT_DIM * TOPK)
N_EXPERTS = 16
N_CORES = 8
N_TOK = 8192
TOK_S = N_TOK // N_CORES   # shared-expert tokens per core
KD = D // P

BF16 = mybir.dt.bfloat16
F32 = mybir.dt.float32
bf16 = ml_dtypes.bfloat16

LAST_EXEC_TIME_NS = None
LAST_RESULTS = None

_prog_cache = {}


def _ensure_ntff_hook():
    """Register the axon NTFF profile hook if the image's antenv lacks it."""
    import sys
    import types
    try:
        from antenv.axon_hooks import get_axon_ntff_profile_hook  # noqa: F401
        return
    except ImportError:
        pass
    try:
        import antenv
        from trn_agent_boot.trn_boot import _ntff_profile_via_ctypes
        hook = _ntff_profile_via_ctypes('/opt/axon/libaxon_pjrt.so')
        mod = types.ModuleType("antenv.axon_hooks")
        mod.get_axon_ntff_profile_hook = lambda: hook
        mod.set_axon_ntff_profile_hook = lambda h: None
        sys.modules["antenv.axon_hooks"] = mod
        antenv.axon_hooks = mod
    except Exception:
        pass


def _ffn(tc, pools, w_u, w_g, w_d, x_dram, y_dram, H, C):
    """Emit one FFN: y = (silu(x.T@wu) * (x.T@wg)) @ wd, feature-major.

    x_dram: [D, C] bf16 (tokens on columns), w_u/w_g: host-tiled
    [H//P, P, KD, P], w_d: [D//P, P, H//P, P], y_dram: [D, C] f32.
    """
    nc = tc.nc
    MH = H // P
    KH = H // P
    MD = D // P
    chunks = [(s, min(512, C - s)) for s in range(0, C, 512)]

    xpool, hpool, wpool, wdpool, hupool, ypool, pspool = pools

    # First weight stripes BEFORE the x loads so the first LDWEIGHTS isn't
    # queued behind 4.7MB of token DMA; x is loaded per-k-tile so the first
    # matmuls start as soon as k-tile 0 lands.
    def load_stripe(pool, w, m, tg):
        sb = pool.tile([P, w.shape[2], P], BF16, tag=tg, name=f"{tg}{m}")
        nc.sync.dma_start(sb[:], w.ap()[m])
        return sb

    wu0 = load_stripe(wpool, w_u, 0, "wu")
    wg0 = load_stripe(wpool, w_g, 0, "wg")

    x_r = x_dram.ap().rearrange("(ko p) c -> ko p c", p=P)
    x_sb = []
    for k in range(KD):
        xk = xpool.tile([P, C], BF16, tag="x")
        nc.sync.dma_start(xk[:], x_r[k])
        x_sb.append(xk)
    # Per-m h tiles so stage B's dependency is per-tile, not whole-h.
    h_sb = [hpool.tile([P, C], BF16, tag="h", name=f"h{m}") for m in range(MH)]

    # Stage A: h = silu(x.T @ wu) * (x.T @ wg), kept in SBUF as bf16
    for m in range(MH):
        wu_sb = wu0 if m == 0 else load_stripe(wpool, w_u, m, "wu")
        wg_sb = wg0 if m == 0 else load_stripe(wpool, w_g, m, "wg")
        for (s, sz) in chunks:
            pu = pspool.tile([P, 512], F32, tag="ps")
            pg = pspool.tile([P, 512], F32, tag="ps")
            for k in range(KD):
                nc.tensor.matmul(pu[:, :sz], wu_sb[:, k], x_sb[k][:, s:s + sz],
                                 start=(k == 0), stop=(k == KD - 1))
            for k in range(KD):
                nc.tensor.matmul(pg[:, :sz], wg_sb[:, k], x_sb[k][:, s:s + sz],
                                 start=(k == 0), stop=(k == KD - 1))
            hu = hupool.tile([P, 512], F32, tag="hu")
            nc.scalar.activation(hu[:, :sz], pu[:, :sz],
                                 mybir.ActivationFunctionType.Silu)
            nc.vector.tensor_mul(h_sb[m][:, s:s + sz], hu[:, :sz], pg[:, :sz])

    # Stage B: y = h @ wd
    y_r = y_dram.ap().rearrange("(mo p) c -> p mo c", p=P)
    for m in range(MD):
        wd_sb = wdpool.tile([P, KH, P], BF16, tag="wd")
        nc.sync.dma_start(wd_sb[:], w_d.ap()[m])
        for (s, sz) in chunks:
            py = pspool.tile([P, 512], F32, tag="ps")
            for k in range(KH):
                nc.tensor.matmul(py[:, :sz], wd_sb[:, k], h_sb[k][:, s:s + sz],
                                 start=(k == 0), stop=(k == KH - 1))
            yo = ypool.tile([P, 512], F32, tag="y")
            nc.vector.tensor_copy(yo[:, :sz], py[:, :sz])
            nc.sync.dma_start(y_r[:, m, s:s + sz], yo[:, :sz])


def _build(C0, C1):
    key = (C0, C1)
    if key in _prog_cache:
        return _prog_cache[key]
    nc = bacc.Bacc("TRN2", target_bir_lowering=False, debug=False)

    t = {}
    for slot, C, H in (("0", C0, H_E), ("1", C1, H_E), ("s", TOK_S, H_S)):
        t[f"xt{slot}"] = nc.dram_tensor(f"xt{slot}", [D, C], BF16,
                                        kind="ExternalInput")
        t[f"wu{slot}"] = nc.dram_tensor(f"wu{slot}", [H // P, P, KD, P], BF16,
                                        kind="ExternalInput")
        t[f"wg{slot}"] = nc.dram_tensor(f"wg{slot}", [H // P, P, KD, P], BF16,
                                        kind="ExternalInput")
        t[f"wd{slot}"] = nc.dram_tensor(f"wd{slot}", [D // P, P, H // P, P], BF16,
                                        kind="ExternalInput")
        t[f"y{slot}"] = nc.dram_tensor(f"y{slot}", [D, C], F32,
                                       kind="ExternalOutput")

    with tile.TileContext(nc) as tc, ExitStack() as ctx:
        pools = (
            ctx.enter_context(tc.tile_pool(name="xpool", bufs=32)),
            ctx.enter_context(tc.tile_pool(name="hpool", bufs=24)),
            ctx.enter_context(tc.tile_pool(name="wpool", bufs=4)),
            ctx.enter_context(tc.tile_pool(name="wdpool", bufs=4)),
            ctx.enter_context(tc.tile_pool(name="hupool", bufs=3)),
            ctx.enter_context(tc.tile_pool(name="ypool", bufs=3)),
            ctx.enter_context(tc.tile_pool(name="ps", bufs=8, space="PSUM")),
        )
        for slot, C, H in (("0", C0, H_E), ("1", C1, H_E), ("s", TOK_S, H_S)):
            _ffn(tc, pools, t[f"wu{slot}"], t[f"wg{slot}"], t[f"wd{slot}"],
                 t[f"xt{slot}"], t[f"y{slot}"], H, C)
    nc.compile()
    _prog_cache[key] = nc
    return nc


def _tile_w(w):
    """[K, M] -> [M//P, P, K//P, P] so each m-stripe is one contiguous slab."""
    K, M = w.shape
    w16 = w.astype(bf16)
    return np.ascontiguousarray(
        w16.reshape(K // P, P, M // P, P).transpose(2, 1, 0, 3))


def _roundup(n, q=32):
    return max(q, ((n + q - 1) // q) * q)


def kernel(x=None, router_w=None, router_bias=None, Wu=None, Wg=None, Wd=None,
           Su=None, Sg=None, Sd=None, _profile=False, _trace_cores=None):
    global LAST_EXEC_TIME_NS, LAST_RESULTS
    flat = np.ascontiguousarray(np.asarray(x, dtype=np.float32).reshape(N_TOK, D))

    # ---- Router on host (float64 ~= exact; selection mirrors jax.lax.top_k) --
    logits = flat.astype(np.float64) @ np.asarray(router_w, np.float64).T
    biased = logits + np.asarray(router_bias, np.float64)[None, :]
    ar = np.arange(N_TOK)
    i1 = np.argmax(biased, axis=1)
    b2 = biased.copy()
    b2[ar, i1] = -np.inf
    i2 = np.argmax(b2, axis=1)
    # gate weights: softmax over all logits (unbiased), renormalized over top-2
    e1 = np.exp(logits[ar, i1] - logits.max(1))
    e2 = np.exp(logits[ar, i2] - logits.max(1))
    w1 = e1 / (e1 + e2)
    w2 = e2 / (e1 + e2)

    # ---- Dispatch: token lists per expert, balanced expert->core pairing ----
    te_idx, te_w = [], []
    for e in range(N_EXPERTS):
        m1 = i1 == e
        m2 = i2 == e
        idx = np.nonzero(m1 | m2)[0]
        w = np.where(m1[idx], w1[idx], w2[idx])
        te_idx.append(idx)
        te_w.append(w)
    counts = np.array([len(ix) for ix in te_idx])
    rank = np.argsort(-counts, kind="stable")
    C0 = _roundup(counts[rank[0]])
    C1 = _roundup(counts[rank[8:]].max())

    nc = _build(C0, C1)

    # ---- Shard inputs per core ----
    XT16 = np.ascontiguousarray(flat.T).astype(bf16)       # [D, N] bf16
    su_t, sg_t = _tile_w(np.asarray(Su)[0]), _tile_w(np.asarray(Sg)[0])
    sd_t = _tile_w(np.asarray(Sd)[0])
    in_maps = []
    core_experts = []
    for c in range(N_CORES):
        e0, e1_ = int(rank[c]), int(rank[15 - c])
        core_experts.append((e0, e1_))
        im = {}
        for slot, e, C in (("0", e0, C0), ("1", e1_, C1)):
            idx = te_idx[e]
            xe = np.zeros((D, C), bf16)
            xe[:, :len(idx)] = XT16[:, idx]
            im[f"xt{slot}"] = xe
            im[f"wu{slot}"] = _tile_w(np.asarray(Wu)[e])
            im[f"wg{slot}"] = _tile_w(np.asarray(Wg)[e])
            im[f"wd{slot}"] = _tile_w(np.asarray(Wd)[e])
        im["xts"] = np.ascontiguousarray(XT16[:, c * TOK_S:(c + 1) * TOK_S])
        im["wus"], im["wgs"], im["wds"] = su_t, sg_t, sd_t
        in_maps.append(im)

    # ---- Run on 8 NeuronCores ----
    if _profile:
        _ensure_ntff_hook()
    res = run_bass_kernel_spmd(
        nc, in_maps, list(range(N_CORES)),
        trace=bool(_profile),
        trace_cores=_trace_cores,
    )
    LAST_EXEC_TIME_NS = res.exec_time_ns
    LAST_RESULTS = res

    # ---- Combine: weighted scatter-add + shared expert ----
    out = np.zeros((N_TOK, D), np.float32)
    for c in range(N_CORES):
        r = res.results[c]
        for slot, e in (("0", core_experts[c][0]), ("1", core_experts[c][1])):
            idx = te_idx[e]
            w = te_w[e].astype(np.float32)
            y = r[f"y{slot}"]                       # [D, C] f32
            out[idx, :] += w[:, None] * y[:, :len(idx)].T
        out[c * TOK_S:(c + 1) * TOK_S, :] += r["ys"].T
    return out.reshape(4, 2048, D)


# revision 10
# speedup vs baseline: 1.0250x; 1.0250x over previous
"""MoE FFN kernel for 8 Trainium2 NeuronCores (expert-parallel).

Strategy:
  - Host computes the router (float64) and dispatches tokens by top-2
    assignment. Experts are sorted by token count and paired
    (rank i, rank 15-i) onto core i, so per-core work is balanced.
  - Each core runs 3 FFN instances: its two routed experts (hidden 1024)
    over their dispatched tokens, and the shared expert (hidden 2048) over
    a 1/8 token slice.  All matmuls in bf16 with fp32 PSUM accumulation;
    tokens live on the matmul free dim so no transposes are needed.
  - Gate weights are folded into the host-side scatter-add combine.
"""

import numpy as np
import ml_dtypes
from contextlib import ExitStack

import concourse.mybir as mybir
import concourse.tile as tile
from concourse import bacc
from concourse.bass_utils import run_bass_kernel_spmd

P = 128
D = 2048
H_E = 1024           # expert hidden dim
H_S = 2048           # shared expert hidden dim (EXPERT_DIM * TOPK)
N_EXPERTS = 16
N_CORES = 8
N_TOK = 8192
TOK_S = N_TOK // N_CORES   # shared-expert tokens per core
KD = D // P

BF16 = mybir.dt.bfloat16
F32 = mybir.dt.float32
bf16 = ml_dtypes.bfloat16

LAST_EXEC_TIME_NS = None
LAST_RESULTS = None

_prog_cache = {}


def _ensure_ntff_hook():
    """Register the axon NTFF profile hook if the image's antenv lacks it."""
    import sys
    import types
    try:
        from antenv.axon_hooks import get_axon_ntff_profile_hook  # noqa: F401
        return
    except ImportError:
        pass
    try:
        import antenv
        from trn_agent_boot.trn_boot import _ntff_profile_via_ctypes
        hook = _ntff_profile_via_ctypes('/opt/axon/libaxon_pjrt.so')
        mod = types.ModuleType("antenv.axon_hooks")
        mod.get_axon_ntff_profile_hook = lambda: hook
        mod.set_axon_ntff_profile_hook = lambda h: None
        sys.modules["antenv.axon_hooks"] = mod
        antenv.axon_hooks = mod
    except Exception:
        pass


def _chunks(C):
    return [(s, min(512, C - s)) for s in range(0, C, 512)]


def _load_stripe(nc, pool, w, m, tg):
    sb = pool.tile([P, w.shape[2], P], BF16, tag=tg, name=f"{tg}{m}")
    nc.sync.dma_start(sb[:], w.ap()[m])
    return sb


def _emit_x(nc, xpool, x_dram, C, slot):
    """Per-k-tile x loads so matmuls start as soon as k-tile 0 lands."""
    x_r = x_dram.ap().rearrange("(ko p) c -> ko p c", p=P)
    x_sb = []
    for k in range(KD):
        xk = xpool.tile([P, C], BF16, tag="x", name=f"x{slot}_{k}")
        nc.sync.dma_start(xk[:], x_r[k])
        x_sb.append(xk)
    return x_sb


def _stage_a(tc, pools, w_u, w_g, x_sb, first_stripes, H, C, slot):
    """h = silu(x.T @ wu) * (x.T @ wg), kept in SBUF as bf16 (per-m tiles)."""
    nc = tc.nc
    MH = H // P
    xpool, hpool, wpool, wdpool, hupool, ypool, pspool = pools
    h_sb = [hpool.tile([P, C], BF16, tag="h", name=f"h{slot}_{m}")
            for m in range(MH)]
    for m in range(MH):
        if m == 0 and first_stripes is not None:
            wu_sb, wg_sb = first_stripes
        else:
            wu_sb = _load_stripe(nc, wpool, w_u, m, "wu")
            wg_sb = _load_stripe(nc, wpool, w_g, m, "wg")
        for (s, sz) in _chunks(C):
            pu = pspool.tile([P, 512], F32, tag="ps", name="pu")
            pg = pspool.tile([P, 512], F32, tag="ps", name="pg")
            for k in range(KD):
                nc.tensor.matmul(pu[:, :sz], wu_sb[:, k], x_sb[k][:, s:s + sz],
                                 start=(k == 0), stop=(k == KD - 1))
            for k in range(KD):
                nc.tensor.matmul(pg[:, :sz], wg_sb[:, k], x_sb[k][:, s:s + sz],
                                 start=(k == 0), stop=(k == KD - 1))
            hu = hupool.tile([P, 512], F32, tag="hu", name="hu")
            nc.scalar.activation(hu[:, :sz], pu[:, :sz],
                                 mybir.ActivationFunctionType.Silu)
            nc.vector.tensor_mul(h_sb[m][:, s:s + sz], hu[:, :sz], pg[:, :sz])
    return h_sb


def _stage_b(tc, pools, w_d, h_sb, y_dram, H, C):
    """y = h @ wd (bf16 output)."""
    nc = tc.nc
    KH = H // P
    MD = D // P
    xpool, hpool, wpool, wdpool, hupool, ypool, pspool = pools
    y_r = y_dram.ap().rearrange("(mo p) c -> p mo c", p=P)
    for m in range(MD):
        wd_sb = _load_stripe(nc, wdpool, w_d, m, "wd")
        for (s, sz) in _chunks(C):
            py = pspool.tile([P, 512], F32, tag="ps", name="py")
            for k in range(KH):
                nc.tensor.matmul(py[:, :sz], wd_sb[:, k], h_sb[k][:, s:s + sz],
                                 start=(k == 0), stop=(k == KH - 1))
            yo = ypool.tile([P, 512], BF16, tag="y", name="yo")
            nc.vector.tensor_copy(yo[:, :sz], py[:, :sz])
            nc.sync.dma_start(y_r[:, m, s:s + sz], yo[:, :sz])


def _build(C0, C1):
    key = (C0, C1)
    if key in _prog_cache:
        return _prog_cache[key]
    nc = bacc.Bacc("TRN2", target_bir_lowering=False, debug=False)

    ffns = [("0", C0, H_E), ("1", C1, H_E), ("s", TOK_S, H_S)]
    t = {}
    for slot, C, H in ffns:
        t[f"xt{slot}"] = nc.dram_tensor(f"xt{slot}", [D, C], BF16,
                                        kind="ExternalInput")
        t[f"wu{slot}"] = nc.dram_tensor(f"wu{slot}", [H // P, P, KD, P], BF16,
                                        kind="ExternalInput")
        t[f"wg{slot}"] = nc.dram_tensor(f"wg{slot}", [H // P, P, KD, P], BF16,
                                        kind="ExternalInput")
        t[f"wd{slot}"] = nc.dram_tensor(f"wd{slot}", [D // P, P, H // P, P], BF16,
                                        kind="ExternalInput")
        t[f"y{slot}"] = nc.dram_tensor(f"y{slot}", [D, C], BF16,
                                       kind="ExternalOutput")

    with tile.TileContext(nc) as tc, ExitStack() as ctx:
        pools = (
            ctx.enter_context(tc.tile_pool(name="xpool", bufs=32)),
            ctx.enter_context(tc.tile_pool(name="hpool", bufs=24)),
            ctx.enter_context(tc.tile_pool(name="wpool", bufs=4)),
            ctx.enter_context(tc.tile_pool(name="wdpool", bufs=4)),
            ctx.enter_context(tc.tile_pool(name="hupool", bufs=3)),
            ctx.enter_context(tc.tile_pool(name="ypool", bufs=3)),
            ctx.enter_context(tc.tile_pool(name="ps", bufs=8, space="PSUM")),
        )
        xpool, wpool = pools[0], pools[2]

        # FFN 0: first weight stripes, then x (so LDWEIGHTS isn't queued
        # behind the token DMA).
        stripes = {"0": (_load_stripe(nc, wpool, t["wu0"], 0, "wu"),
                         _load_stripe(nc, wpool, t["wg0"], 0, "wg"))}
        xs = {"0": _emit_x(nc, xpool, t["xt0"], C0, "0")}

        for i, (slot, C, H) in enumerate(ffns):
            h_sb = _stage_a(tc, pools, t[f"wu{slot}"], t[f"wg{slot}"],
                            xs[slot], stripes.get(slot), H, C, slot)
            # Prefetch the next FFN's tokens + first stripes during this
            # FFN's stage-A compute window (stage B's DMA is the busy one).
            if i + 1 < len(ffns):
                ns, nC, nH = ffns[i + 1]
                stripes[ns] = (_load_stripe(nc, wpool, t[f"wu{ns}"], 0, "wu"),
                               _load_stripe(nc, wpool, t[f"wg{ns}"], 0, "wg"))
                xs[ns] = _emit_x(nc, xpool, t[f"xt{ns}"], nC, ns)
            _stage_b(tc, pools, t[f"wd{slot}"], h_sb, t[f"y{slot}"], H, C)
    nc.compile()
    _prog_cache[key] = nc
    return nc


def _tile_w(w):
    """[K, M] -> [M//P, P, K//P, P] so each m-stripe is one contiguous slab."""
    K, M = w.shape
    w16 = w.astype(bf16)
    return np.ascontiguousarray(
        w16.reshape(K // P, P, M // P, P).transpose(2, 1, 0, 3))


def _roundup(n, q=32):
    return max(q, ((n + q - 1) // q) * q)


def kernel(x=None, router_w=None, router_bias=None, Wu=None, Wg=None, Wd=None,
           Su=None, Sg=None, Sd=None, _profile=False, _trace_cores=None):
    global LAST_EXEC_TIME_NS, LAST_RESULTS
    flat = np.ascontiguousarray(np.asarray(x, dtype=np.float32).reshape(N_TOK, D))

    # ---- Router on host (float64 ~= exact; selection mirrors jax.lax.top_k) --
    logits = flat.astype(np.float64) @ np.asarray(router_w, np.float64).T
    biased = logits + np.asarray(router_bias, np.float64)[None, :]
    ar = np.arange(N_TOK)
    i1 = np.argmax(biased, axis=1)
    b2 = biased.copy()
    b2[ar, i1] = -np.inf
    i2 = np.argmax(b2, axis=1)
    # gate weights: softmax over all logits (unbiased), renormalized over top-2
    e1 = np.exp(logits[ar, i1] - logits.max(1))
    e2 = np.exp(logits[ar, i2] - logits.max(1))
    w1 = e1 / (e1 + e2)
    w2 = e2 / (e1 + e2)

    # ---- Dispatch: token lists per expert, balanced expert->core pairing ----
    te_idx, te_w = [], []
    for e in range(N_EXPERTS):
        m1 = i1 == e
        m2 = i2 == e
        idx = np.nonzero(m1 | m2)[0]
        w = np.where(m1[idx], w1[idx], w2[idx])
        te_idx.append(idx)
        te_w.append(w)
    counts = np.array([len(ix) for ix in te_idx])
    rank = np.argsort(-counts, kind="stable")
    C0 = _roundup(counts[rank[0]])
    C1 = _roundup(counts[rank[8:]].max())

    nc = _build(C0, C1)

    # ---- Shard inputs per core ----
    XT16 = np.ascontiguousarray(flat.T).astype(bf16)       # [D, N] bf16
    su_t, sg_t = _tile_w(np.asarray(Su)[0]), _tile_w(np.asarray(Sg)[0])
    sd_t = _tile_w(np.asarray(Sd)[0])
    in_maps = []
    core_experts = []
    for c in range(N_CORES):
        e0, e1_ = int(rank[c]), int(rank[15 - c])
        core_experts.append((e0, e1_))
        im = {}
        for slot, e, C in (("0", e0, C0), ("1", e1_, C1)):
            idx = te_idx[e]
            xe = np.zeros((D, C), bf16)
            xe[:, :len(idx)] = XT16[:, idx]
            im[f"xt{slot}"] = xe
            im[f"wu{slot}"] = _tile_w(np.asarray(Wu)[e])
            im[f"wg{slot}"] = _tile_w(np.asarray(Wg)[e])
            im[f"wd{slot}"] = _tile_w(np.asarray(Wd)[e])
        im["xts"] = np.ascontiguousarray(XT16[:, c * TOK_S:(c + 1) * TOK_S])
        im["wus"], im["wgs"], im["wds"] = su_t, sg_t, sd_t
        in_maps.append(im)

    # ---- Run on 8 NeuronCores ----
    if _profile:
        _ensure_ntff_hook()
    res = run_bass_kernel_spmd(
        nc, in_maps, list(range(N_CORES)),
        trace=bool(_profile),
        trace_cores=_trace_cores,
    )
    LAST_EXEC_TIME_NS = res.exec_time_ns
    LAST_RESULTS = res

    # ---- Combine: weighted scatter-add + shared expert ----
    out = np.zeros((N_TOK, D), np.float32)
    for c in range(N_CORES):
        r = res.results[c]
        for slot, e in (("0", core_experts[c][0]), ("1", core_experts[c][1])):
            idx = te_idx[e]
            w = te_w[e].astype(np.float32)
            y = r[f"y{slot}"].astype(np.float32)     # [D, C] bf16 -> f32
            out[idx, :] += w[:, None] * y[:, :len(idx)].T
        out[c * TOK_S:(c + 1) * TOK_S, :] += r["ys"].astype(np.float32).T
    return out.reshape(4, 2048, D)
